# revision 29
# baseline (speedup 1.0000x reference)
"""Trainium2 Bass kernel for modulated multi-head attention (q=k=v variant).

v2 — restructured from the v1 baseline using HW calibration:
  * All weight modulation/demodulation (style matvec, w*style, rsqrt demod,
    for both k- and o-projections) is precomputed on HOST in fp32 and shipped
    as per-batch effective weight matrices (bf16). The device only runs:
      kqvT = wk_eff @ x^T              [F-part, N]   (q/k source, demodulated)
      kqv_v = x @ wk_eff^T             [N-part, F]   (v source, demodulated)
      per head h: S_h = q_h q_h^T/8 ; E=exp(S) with fused rowsum (accum_out)
      outT_h = v_h^T E_h               (attnv, PSUM-accumulated over m-blocks)
      aT = outT * (1/rowsum) broadcast (DRAM-bounce broadcast as in v1)
      y = aT^T @ wo_eff                (pair-merged K=128 projection)
  * HW calibration findings applied:
      - consecutive matmuls must not alternate PE tile configs
        (tile_position / stationary size); poison costs ~4us per switch.
        All matmul streams are batched per config (scores h0-batch, h1-batch,
        attnv h0/h1 batches per half-pair).
      - accumulating matmul groups must alternate PSUM banks between
        consecutive instructions (same-bank back-to-back is ~5x slow);
        kqvT / kqv_v / yproj groups are emitted pairwise bank-interleaved.
      - exp (FD=1024, accum_out) sustains ~1.05us when paced cross-engine
        with rotating PSUM sources; scores tiles rotate 3 slots.
  * exp order per pair: all 8 h0-exps (first half), then 8 h1-exps — this
    makes every PE stream a clean per-config batch.
  * PSUM budget (8 banks): "sc" [128,1024]x3 slots (6 banks; scores tiles and
    scratch for kqvT/kqv_v/yproj groups) + accA/accB [128,512] (2 banks,
    attnv accumulators: h0 rows 0:64 cfg (0,0), h1 rows 64:128 cfg (0,64)).

Sharding: data-parallel over batch B=8, one batch element per NeuronCore.
"""

import sys

if "/opt/trn_rl_repo" not in sys.path:
    sys.path.insert(0, "/opt/trn_rl_repo")

from contextlib import ExitStack

import numpy as np

import concourse.bass as bass
import concourse.bacc as bacc
import concourse.mybir as mybir
import concourse.tile as tile
from concourse.bass_utils import run_bass_kernel_spmd

P = 128          # partitions
F = 512          # hidden dim
C4 = F // P      # 4 feature chunks of 128
N = 1024         # tokens
NB = N // P      # 8 token blocks
H = 8            # heads
D = 64           # head dim
B = 8            # batch (one per core)
SCALE = 1.0 / 8.0   # 1/sqrt(D)
EPS = 1e-8

F32 = mybir.dt.float32
BF16 = mybir.dt.bfloat16


def _bcast(ap_1d, parts):
    """Partition-broadcast read AP for a 1-D DRAM AP."""
    return bass.AP(
        tensor=ap_1d.tensor,
        offset=ap_1d.offset,
        ap=[[0, parts]] + [list(d) for d in ap_1d.ap],
    )


def _emit(nc, loop_reps=0, lvl=4):
    xT = nc.dram_tensor("xT", [F, N], BF16, kind="ExternalInput")
    wkT = nc.dram_tensor("wkT", [F, F], BF16, kind="ExternalInput")
    woT = nc.dram_tensor("woT", [F, F], BF16, kind="ExternalInput")
    y = nc.dram_tensor("y", [N, F], F32, kind="ExternalOutput")

    with tile.TileContext(nc) as tc:
        if loop_reps:
            with tc.For_i(0, loop_reps, 1):
                _emit_body(nc, tc, xT, wkT, woT, y, lvl=lvl)
        else:
            _emit_body(nc, tc, xT, wkT, woT, y, lvl=lvl)


def _emit_body(nc, tc, xT, wkT, woT, y, lvl=4):
    f32 = F32
    Exp = mybir.ActivationFunctionType.Exp
    MULT = mybir.AluOpType.mult

    with ExitStack() as ctx:
        persist = ctx.enter_context(tc.tile_pool(name="persist", bufs=1))
        dram = ctx.enter_context(tc.tile_pool(name="dram", bufs=2, space="DRAM"))
        psum = ctx.enter_context(tc.tile_pool(name="psum", bufs=1, space="PSUM"))
        att = ctx.enter_context(tc.tile_pool(name="att", bufs=1))
        attrs = ctx.enter_context(tc.tile_pool(name="attrs", bufs=2))

        # ---- persistent SBUF tiles ----
        xT_sb = persist.tile([P, C4, N], BF16)
        wk_sb = persist.tile([P, C4, F], BF16)
        # head-sliced layouts (all on partitions 0:64) so attnv and yproj
        # never need col tile_position
        wo_sb = persist.tile([D, H, F], BF16)
        kqvT = persist.tile([P, C4, N], BF16)
        kqv_v = persist.tile([P, NB, F], BF16)
        aT = persist.tile([D, H, N], BF16)
        y_acc = persist.tile([P, NB, F], f32)

        # exp-table prewarm while input DMAs stream
        warm = persist.tile([1, 1], f32)
        nc.vector.memset(warm, 1.0)
        nc.scalar.activation(out=warm, in_=warm, func=Exp, scale=1.0)

        # ---- input DMAs ----
        xT_r = xT.rearrange("(c p) n -> p c n", p=P)
        for nh in range(2):
            nc.sync.dma_start(out=xT_sb[:, :, nh * F : (nh + 1) * F],
                              in_=xT_r[:, :, nh * F : (nh + 1) * F])
        nc.gpsimd.dma_start(out=wk_sb, in_=wkT.rearrange("(c p) o -> p c o", p=P))

        def sc_tile():
            return psum.tile([P, N], f32, tag="sc", bufs=2, name="sc")

        # ---- kqvT chunk: features chunk ob (pair ob), both n-halves ----
        def emit_kqvT_chunk(ob):
            pt = sc_tile()
            for c in range(C4):
                for nh in range(2):
                    nc.tensor.matmul(
                        pt[:, nh * F : (nh + 1) * F],
                        wk_sb[:, c, ob * P : (ob + 1) * P],
                        xT_sb[:, c, nh * F : (nh + 1) * F],
                        start=(c == 0), stop=(c == C4 - 1))
            nc.vector.tensor_copy(out=kqvT[:, ob, :], in_=pt)

        # ---- kqv_v: two n-blocks at a time, banks interleaved ----
        def emit_kqv_v_pair(nb0):
            pt = sc_tile()
            for c in range(C4):
                for k in range(2):
                    nc.tensor.matmul(
                        pt[:, k * F : (k + 1) * F],
                        xT_sb[:, c, (nb0 + k) * P : (nb0 + k + 1) * P],
                        wk_sb[:, c, :],
                        start=(c == 0), stop=(c == C4 - 1))
            nc.vector.tensor_copy(out=kqv_v[:, nb0, :], in_=pt[:, 0:F])
            nc.vector.tensor_copy(out=kqv_v[:, nb0 + 1, :], in_=pt[:, F:N])

        # ---- y projection partial for pair pc: per-head K=64 accumulating
        # pairs (no col tile_position anywhere), two blocks per sc slot,
        # merged [128,1024] y-add and merged 2-block output DMA ----
        def emit_ypartial(pc, nbs, with_dma):
            for i in range(0, len(nbs), 2):
                pt = sc_tile()
                for k in range(2):
                    nb = nbs[i + k]
                    for hh in range(2):
                        h = 2 * pc + hh
                        nc.tensor.matmul(
                            pt[:, k * F : (k + 1) * F],
                            aT[:, h, nb * P : (nb + 1) * P],
                            wo_sb[:, h, :],
                            start=(hh == 0), stop=(hh == 1))
                nb0 = nbs[i]
                if pc == 0:
                    nc.vector.tensor_copy(out=y_acc[:, nb0 : nb0 + 2, :],
                                          in_=pt)
                else:
                    nc.vector.tensor_add(out=y_acc[:, nb0 : nb0 + 2, :],
                                         in0=y_acc[:, nb0 : nb0 + 2, :],
                                         in1=pt)
                if with_dma:
                    eng = nc.sync if nb0 % 4 == 0 else nc.gpsimd
                    eng.dma_start(
                        out=y.rearrange("(b p) f -> p b f", p=P)[:, nb0 : nb0 + 2, :],
                        in_=y_acc[:, nb0 : nb0 + 2, :])

        emit_kqvT_chunk(0)

        # per-pair state carried across the pair loop
        pair_state = {}

        def emit_scores_batch(pc, hh, E, rows, mbs):
            """Batch of scores MMs + exps for head-half hh (0: rows 0:64,
            1: rows 64:128 with tile_position) of pair pc."""
            lo, hi = (0, D) if hh == 0 else (D, P)
            kw = {} if hh == 0 else {"tile_position": (64, 0)}
            for mb in mbs:
                s = sc_tile()
                for nh in range(2):
                    nc.tensor.matmul(
                        s[:, nh * F : (nh + 1) * F],
                        kqvT[lo:hi, pc, mb * P : (mb + 1) * P],
                        kqvT[lo:hi, pc, nh * F : (nh + 1) * F],
                        start=True, stop=True, **kw)
                nc.scalar.activation(out=E[:, mb, :], in_=s, func=Exp,
                                     scale=SCALE,
                                     accum_out=rows[:, mb : mb + 1])

        def emit_attnv_batch(pc, hh, mbs):
            """attnv for head 2*pc+hh: stationary v cols 0:64, output
            partitions 0:64, own acc bank pair per head - no tile_position."""
            h = 2 * pc + hh
            st = pair_state[pc]
            E = st["E0" if hh == 0 else "E1"]
            accs = (st["accA"], st["accB"]) if hh == 0 else (st["accC"], st["accD"])
            for mb in mbs:
                first, last = mb == mbs[0], mb == mbs[-1]
                for nh, acc in ((0, accs[0]), (1, accs[1])):
                    nc.tensor.matmul(
                        acc,
                        kqv_v[:, mb, h * D : (h + 1) * D],
                        E[:, mb, nh * F : (nh + 1) * F],
                        start=first, stop=last)

        def emit_finish_half(pc, hh):
            """one head's rowsum reciprocal -> DRAM-bounce broadcast"""
            st = pair_state[pc]
            rows = st["rows0" if hh == 0 else "rows1"]
            nc.vector.reciprocal(out=rows, in_=rows)
            d_r = dram.tile([N], f32, tag=f"d_r{hh}", name="d_r")
            eng = nc.sync if hh == 0 else nc.gpsimd
            eng.dma_start(out=d_r.rearrange("(c p) -> p c", p=P), in_=rows)
            rs_b = attrs.tile([D, N], f32, tag=f"rs_b{hh}", name="rs_b")
            eng.dma_start(out=rs_b, in_=_bcast(d_r, D))
            st[f"rs_b{hh}"] = rs_b

        def emit_pair_finish(pc):
            emit_finish_half(pc, 0)
            emit_finish_half(pc, 1)

        def emit_evac(pc, last):
            st = pair_state[pc]
            for nh in range(2):
                sl = slice(nh * F, (nh + 1) * F)
                accE = st["accA" if nh == 0 else "accB"]
                accO = st["accC" if nh == 0 else "accD"]
                nc.vector.tensor_tensor(aT[:, 2 * pc, sl], accE,
                                        st["rs_b0"][:, sl], MULT)
                nc.vector.tensor_tensor(aT[:, 2 * pc + 1, sl], accO,
                                        st["rs_b1"][:, sl], MULT)
                if last:
                    emit_ypartial(pc, [nh * 4 + j for j in range(4)],
                                  with_dma=True)

        # =================== pair loop ===================
        for pc in range(H // 2):
            E0 = att.tile([P, NB, N], BF16, tag="E0", bufs=2)
            E1 = att.tile([P, NB, N], BF16, tag="E1", bufs=2)
            rows0 = attrs.tile([P, NB], f32, tag="rows0")
            rows1 = attrs.tile([P, NB], f32, tag="rows1")
            accA = psum.tile([D, F], f32, tag="accA")
            accB = psum.tile([D, F], f32, tag="accB")
            accC = psum.tile([D, F], f32, tag="accC")
            accD = psum.tile([D, F], f32, tag="accD")
            pair_state[pc] = dict(E0=E0, E1=E1, rows0=rows0, rows1=rows1,
                                  accA=accA, accB=accB, accC=accC, accD=accD)

            # ----- first half: h0 scores+exps, then a batch of other work,
            # then attnv h1-batch of the previous pair -----
            emit_scores_batch(pc, 0, E0, rows0, list(range(NB)))
            if pc == 0:
                emit_kqvT_chunk(1)
                emit_kqvT_chunk(2)
            else:
                emit_attnv_batch(pc - 1, 1, list(range(NB)))
                emit_pair_finish(pc - 1)
                emit_evac(pc - 1, last=False)

            # ----- second half: h1 scores+exps + other work + attnv h0 -----
            emit_scores_batch(pc, 1, E1, rows1, list(range(NB)))
            if pc == 0:
                emit_kqvT_chunk(3)
                for nb0 in range(0, NB, 2):
                    emit_kqv_v_pair(nb0)
                nc.gpsimd.dma_start(
                    out=wo_sb, in_=woT.rearrange("(h d) o -> d h o", d=D))
            elif pc >= 2:
                if pc == 3:
                    # pair 3's h0 rowsums are final after its first half:
                    # launch that bounce now so the tail never waits on it
                    emit_finish_half(3, 0)
                emit_ypartial(pc - 2, list(range(NB)), with_dma=False)
                if pc == 3:
                    emit_ypartial(2, list(range(NB)), with_dma=False)
            emit_attnv_batch(pc, 0, list(range(NB)))

        # =================== tail ===================
        pc = H // 2 - 1
        emit_finish_half(pc, 1)
        emit_attnv_batch(pc, 1, list(range(NB)))
        emit_evac(pc, last=True)


_NC_CACHE = None


def build_nc():
    global _NC_CACHE
    if _NC_CACHE is None:
        nc = bacc.Bacc(trn_type="TRN2")
        _emit(nc)
        nc.finalize()
        _NC_CACHE = nc
    return _NC_CACHE


def _eff_weightT(weight, style):
    """Host: modulated+demodulated weight, transposed, bf16.
    weight [O, I] fp32, style [I] fp32 -> [I, O] bf16."""
    import ml_dtypes
    w = weight * style[None, :]
    w = w * (1.0 / np.sqrt((w * w).sum(axis=1) + EPS))[:, None]
    return np.ascontiguousarray(w.T.astype(ml_dtypes.bfloat16))


def make_in_maps(x, s, k_weight, k_aff_w, k_aff_b, o_weight, o_aff_w, o_aff_b):
    import ml_dtypes
    f = np.float32
    bf = ml_dtypes.bfloat16
    x = np.asarray(x, f)
    s = np.asarray(s, f)
    k_weight = np.asarray(k_weight, f)
    k_aff_w = np.asarray(k_aff_w, f)
    k_aff_b = np.asarray(k_aff_b, f)
    o_weight = np.asarray(o_weight, f)
    o_aff_w = np.asarray(o_aff_w, f)
    o_aff_b = np.asarray(o_aff_b, f)
    in_maps = []
    for b in range(B):
        style_k = s[b] @ k_aff_w.T + k_aff_b
        style_o = s[b] @ o_aff_w.T + o_aff_b
        in_maps.append({
            "xT": np.ascontiguousarray(x[b].T.astype(bf)),
            "wkT": _eff_weightT(k_weight, style_k),
            "woT": _eff_weightT(o_weight, style_o),
        })
    return in_maps


def kernel(x, s, k_weight, k_aff_w, k_aff_b, o_weight, o_aff_w, o_aff_b):
    assert x.shape == (B, N, F), x.shape
    nc = build_nc()
    in_maps = make_in_maps(x, s, k_weight, k_aff_w, k_aff_b,
                           o_weight, o_aff_w, o_aff_b)
    res = run_bass_kernel_spmd(nc, in_maps, list(range(B)))
    return np.stack([res.results[b]["y"] for b in range(B)], axis=0)
